# revision 18
# baseline (speedup 1.0000x reference)
"""Trainium2 Bass kernel for sliding-window causal self-attention.

Reference computation (B=1, T=4096, H=8 heads, head_dim=128, DIM=1024):
  qkv = x @ w_qkv.T; q,k = rms_norm -> rope; v = lam0*rms_norm(v) + lam1*ve
  scores = (q k^T) * 0.12 with sliding-window causal mask (0 <= i-j < 512)
  y = softmax(scores) @ v;  out = y @ o_w.T

Sharding over 8 cores: 2 sequence halves (S) x 4 head-pair groups (G).
Core c = 4*s + g handles t in [2048s, 2048(s+1)) for heads {2g, 2g+1}.
Each core reads its x rows plus a 512-row halo of preceding rows (for k/v),
computes its partial output projection over its 2 heads, and the host sums
the 4 partials per half and concatenates the halves.

All tensors travel and compute in fp16 (f32 PSUM accumulation), which halves
DMA and doubles DVE throughput at matmul rates equal to f32r. q/k transposes
to [dd, t] run on the DMA XBAR (dma_start_transpose) instead of the PE,
eliminating both the PE transpose matmuls and their PSUM evacuation copies.
RMS stats use ACT Square+accum and a single Rsqrt (lam0 folded via scale/bias
for v); ve arrives pre-scaled by lam1 so v = psum*rs + ve is one fused
scalar_tensor_tensor. RoPE is 3 DVE ops via a sign-folded [cos|cos]/[sin|-sin]
table and a negative-stride swapped view; the rotated dim pairs are made
adjacent by permuting q/k weight rows host-side (scores are invariant).
Attention uses [kj, qi] scores over 128-query tiles (fp16 matmuls are full
rate at 128 moving), exp has bias -5 so unnormalized probs fit fp16 range,
kj-sums run on the PE with a ones vector, and 1/sum is partition-broadcast
on GpSimd. The scores->exp->weighted-sum chain is software-pipelined so the
PE never waits on the ACT exp. The output projection is interleaved into the
attention loop; its PSUM is evacuated fp16 (split ACT/DVE) and DMA'd out.
"""

import sys

sys.path.insert(0, "/opt/trn_rl_repo")

import numpy as np

import concourse.bass as bass
import concourse.mybir as mybir
import concourse.tile as tile
from concourse import bacc

# Problem constants
T = 4096
DIM = 1024
H = 8
HD = 128
WINDOW = 512
ATTN_SCALE = 0.12
ROPE_BASE = 1024.0
EPS = 1e-6
EXP_BIAS = -5.0

# Sharding
S = 2          # sequence halves
G = 4          # head groups (2 heads each)
HPC = H // G   # heads per core = 2
TC = T // S    # own rows per core = 2048
TK = TC + WINDOW  # k/v rows incl. halo = 2560
NQT = TC // 128   # q tiles per head = 16
NKC = TK // 128   # k chunks = 20
EW = 3 * HPC * HD  # fused qkv width per core = 768

F32 = mybir.dt.float32
F16 = mybir.dt.float16

AOP = mybir.AluOpType
AF = mybir.ActivationFunctionType


def declare_params(nc, out_dtype=F16):
    return {
        "xT": nc.declare_dram_parameter("xT", [DIM, TK], F16, isOutput=False),
        "wqkvT": nc.declare_dram_parameter("wqkvT", [DIM, EW], F16, isOutput=False),
        "woT": nc.declare_dram_parameter("woT", [HPC * HD, DIM], F16, isOutput=False),
        "ve": nc.declare_dram_parameter("ve", [TK, HPC * HD], F16, isOutput=False),
        "rope_tbl": nc.declare_dram_parameter("rope_tbl", [TK, 128], F16, isOutput=False),
        "lam": nc.declare_dram_parameter("lam", [128, 2], F32, isOutput=False),
        "padcnt": nc.declare_dram_parameter("padcnt", [TC], F32, isOutput=False),
        "outT": nc.declare_dram_parameter("outT", [DIM, TC], out_dtype, isOutput=True),
    }


def build_kernel():
    nc = bacc.Bacc()
    p = declare_params(nc)
    with tile.TileContext(nc) as tc:
        _trace_body(nc, tc, **p)
    nc.compile()
    return nc


def _trace_body(nc, tc, xT, wqkvT, woT, ve, rope_tbl, lam, padcnt, outT):
    import contextlib

    ctx = contextlib.ExitStack()
    with ctx:
        const = ctx.enter_context(tc.tile_pool(name="const", bufs=1))
        persist = ctx.enter_context(tc.tile_pool(name="persist", bufs=1))

        # ---- constants needed by phase A (w split per d-chunk so the
        # first projection matmuls can start early) ----
        w_sb = const.tile([128, 8, EW], F16)  # wqkvT as [dpart, dchunk, e]
        wq_r = wqkvT.rearrange("(a p) e -> p a e", p=128)
        nc.sync.dma_start(out=w_sb[:, 0:4, :], in_=wq_r[:, 0:4, :])
        # rope table [t, 128] = [cos|cos|sin|-sin] (32 freqs each)
        rt_sb = const.tile([128, NKC, 128], F16)
        nc.sync.dma_start(
            out=rt_sb, in_=rope_tbl.rearrange("(a p) f -> p a f", p=128))
        lam_sb = const.tile([128, 2], F32)
        nc.sync.dma_start(out=lam_sb, in_=lam[:])

        # ---- B/C constant tiles (DMAs deferred until after first x block) ----
        wo_sb = const.tile([128, HPC, DIM], F16)  # woT as [ddpart, head, e]
        pad_r = const.tile([1, TC], F32)

        ones_col = const.tile([128, 1], F16)
        nc.vector.memset(ones_col, 1.0)
        eps_sb = const.tile([128, 1], F32)
        nc.vector.memset(eps_sb, EPS)
        ebias_sb = const.tile([128, 1], F32)
        nc.vector.memset(ebias_sb, EXP_BIAS)

        # device-exact value of one padded-key softmax term: the pad rows of
        # x are zero, so each contributes fp16(exp_act(0*scale + EXP_BIAS))
        # to the denominator -- computed HERE with the same instruction so it
        # matches the device exp table bit-for-bit (np.exp does not).
        zz = const.tile([1, 1], F32)
        nc.vector.memset(zz, 0.0)
        e5 = const.tile([1, 1], F16)
        nc.scalar.activation(e5, zz, AF.Exp, scale=ATTN_SCALE,
                             bias=ebias_sb[0:1, :])
        pad2 = const.tile([1, TC], F32)

        # Band masks in [kj, qi] orientation for single-chunk q tiles.
        # maskU (leftmost window chunk): keep kj > qi; maskL (diag): kj <= qi.
        mskf = const.tile([128, 2, 128], F32)
        nc.gpsimd.memset(mskf, 1.0)
        nc.gpsimd.affine_select(
            out=mskf[:, 0, :], in_=mskf[:, 0, :], compare_op=AOP.is_ge,
            fill=0.0, base=-1, channel_multiplier=1, pattern=[[-1, 128]],
        )
        nc.gpsimd.affine_select(
            out=mskf[:, 1, :], in_=mskf[:, 1, :], compare_op=AOP.is_ge,
            fill=0.0, base=0, channel_multiplier=-1, pattern=[[1, 128]],
        )
        masks = const.tile([128, 2, 128], F16)  # [0]=maskU, [1]=maskL
        nc.vector.tensor_copy(out=masks, in_=mskf)

        # ---- persistent activations ----
        # qT/kT: [dd, t] per head; v: [t(kj) part, chunk, dd]; yT: [dd, t].
        qT = [persist.tile([128, TC], F16, name=f"qT{h}") for h in range(HPC)]
        kT = [persist.tile([128, TK], F16, name=f"kT{h}") for h in range(HPC)]
        vbf = [persist.tile([128, NKC, HD], F16, name=f"vbf{h}") for h in range(HPC)]
        yT = [persist.tile([128, TC], F16, name=f"yT{h}") for h in range(HPC)]

        # ================= Phase A: QKV projection + norm/rope =================
        with (
            tc.tile_pool(name="xt_pool", bufs=2) as xt_pool,
            tc.tile_pool(name="ve_pool", bufs=2) as ve_pool,
            tc.tile_pool(name="stageA", bufs=4) as stageA,
            tc.tile_pool(name="smallA", bufs=8) as smallA,
            tc.tile_pool(name="proj_psum", bufs=3, space="PSUM") as proj_psum,
            tc.tile_pool(name="sq_psum", bufs=2, space="PSUM") as sq_psum,
        ):
            xT_r = xT.rearrange("(a p) t -> p a t", p=128)  # [128, 8, TK]
            ve_r = ve.rearrange("(a p) d -> p a d", p=128)  # [128, 20, 256]
            TB = 512  # t rows per x block load
            for tb in range(TK // TB):
                xt = xt_pool.tile([128, 8, TB], F16)
                xsrc = xT_r[:, :, tb * TB:(tb + 1) * TB]
                if tb == 0:
                    # split so compute can start on the first half
                    nc.sync.dma_start(out=xt[:, 0:4, :], in_=xsrc[:, 0:4, :])
                    nc.sync.dma_start(out=xt[:, 4:8, :], in_=xsrc[:, 4:8, :])
                else:
                    nc.sync.dma_start(out=xt, in_=xsrc)
                vet = ve_pool.tile([128, 4, HPC * HD], F16)
                nc.sync.dma_start(out=vet, in_=ve_r[:, tb * 4:(tb + 1) * 4, :])
                if tb == 0:
                    # bulk loads deferred behind the first x block
                    nc.sync.dma_start(out=w_sb[:, 4:8, :], in_=wq_r[:, 4:8, :])
                    nc.sync.dma_start(
                        out=wo_sb,
                        in_=woT.rearrange("(a p) e -> p a e", p=128))
                    nc.sync.dma_start(
                        out=pad_r, in_=padcnt.rearrange("(a t) -> a t", a=1))
                    nc.vector.tensor_tensor(
                        out=pad2, in0=pad_r,
                        in1=e5[0:1, 0:1].to_broadcast([1, TC]), op=AOP.mult)
                for tsub in range(TB // 128):
                    c = tb * (TB // 128) + tsub  # t-chunk index, 0..19
                    s0 = 0 if c >= 4 else 2  # halo rows need only k,v
                    psum = proj_psum.tile([128, 6, HD], F32)
                    psf = psum.rearrange("p s d -> p (s d)")
                    # matmul outputs may not cross a 2KB PSUM bank boundary,
                    # so split the 768-wide output at column 512
                    for dch in range(8):
                        lhsT = xt[:, dch, tsub * 128:(tsub + 1) * 128]
                        nc.tensor.matmul(
                            psf[:, s0 * HD:512], lhsT,
                            w_sb[:, dch, s0 * HD:512],
                            start=(dch == 0), stop=(dch == 7),
                        )
                        nc.tensor.matmul(
                            psf[:, 512:EW], lhsT, w_sb[:, dch, 512:EW],
                            start=(dch == 0), stop=(dch == 7),
                        )

                    # RMS stats: ACT Square with fused per-segment row-sum.
                    ssum = smallA.tile([128, 6], F32)
                    sqscr = sq_psum.tile([128, 128], F32)
                    for sg in range(s0, 6):
                        nc.scalar.activation(
                            sqscr, psum[:, sg, :], AF.Square,
                            accum_out=ssum[:, sg:sg + 1],
                        )
                    # rs = 1/sqrt(mean+eps); v gets lam0 folded via scale/bias
                    rms = smallA.tile([128, 6], F32)
                    nc.scalar.activation(rms[:, s0:4], ssum[:, s0:4], AF.Sqrt,
                                         scale=1.0 / HD, bias=eps_sb)
                    nc.scalar.activation(rms[:, 4:6], ssum[:, 4:6], AF.Sqrt,
                                         scale=lam_sb[:, 0:1],
                                         bias=lam_sb[:, 1:2])
                    rs = smallA.tile([128, 6], F32)
                    nc.vector.reciprocal(rs[:, s0:6], rms[:, s0:6])

                    # normalize q,k -> fp16 staging
                    st = stageA.tile([128, 4, HD], F16)
                    nc.vector.tensor_tensor(
                        out=st[:, s0:4, :], in0=psum[:, s0:4, :],
                        in1=rs[:, s0:4, None].to_broadcast([128, 4 - s0, HD]),
                        op=AOP.mult,
                    )

                    # v = psum_v * rs_v + ve_pre (ve pre-scaled by lam1)
                    for h in range(HPC):
                        nc.vector.scalar_tensor_tensor(
                            out=vbf[h][:, c, :], in0=psum[:, 4 + h, :],
                            scalar=rs[:, 4 + h:5 + h], op0=AOP.mult,
                            in1=vet[:, tsub, h * HD:(h + 1) * HD], op1=AOP.add,
                        )

                    # rope on dims [0:64] (pairs (i, i+32); host permuted the
                    # q/k weight rows so rotated pairs are adjacent):
                    #   y = X*[cos|cos] + Xswap*[sin|-sin]
                    st5 = st.rearrange("p s (a i) -> p s a i", a=4)
                    nseg = 4 - s0
                    X = st5[:, s0:4, 0:2, :]
                    Xsw = st5[:, s0:4, 0:2, :][:, :, ::-1, :]
                    rt4 = rt_sb.rearrange("p c (b i) -> p c b i", b=4)
                    cs = rt4[:, c:c + 1, 0:2, :].to_broadcast([128, nseg, 2, 32])
                    sn = rt4[:, c:c + 1, 2:4, :].to_broadcast([128, nseg, 2, 32])
                    t1 = stageA.tile([128, 4, 2, 32], F16)
                    t2 = stageA.tile([128, 4, 2, 32], F16)
                    nc.vector.tensor_tensor(out=t1[:, s0:4], in0=X, in1=cs,
                                            op=AOP.mult)
                    nc.vector.tensor_tensor(out=t2[:, s0:4], in0=Xsw, in1=sn,
                                            op=AOP.mult)
                    nc.vector.tensor_tensor(out=X, in0=t1[:, s0:4],
                                            in1=t2[:, s0:4], op=AOP.add)

                    # transpose q,k into [dd, t] via the DMA XBAR
                    for h in range(HPC):
                        if c >= 4:
                            nc.sync.dma_start_transpose(
                                out=qT[h][:, (c - 4) * 128:(c - 3) * 128],
                                in_=st[:, h, :])
                        nc.sync.dma_start_transpose(
                            out=kT[h][:, c * 128:(c + 1) * 128],
                            in_=st[:, 2 + h, :])

        # ====== Phase B+C: banded attention with interleaved out-projection ===
        # PSUM (8 banks of 2KB): scores 2 banks x2, yv accum 1x2, kj-sums 1,
        # out-proj 1. A matmul group's start=True zero-marks the whole 2KB
        # bank row, so concurrently-accumulating groups (sums vs yps) MUST
        # live in different banks or the second group's start wipes the
        # first group's partial.
        with (
            tc.tile_pool(name="pm_pool", bufs=3) as pm_pool,
            tc.tile_pool(name="smallB", bufs=8) as smallB,
            tc.tile_pool(name="o_out", bufs=2) as o_out,
            tc.tile_pool(name="sc_psum", bufs=2, space="PSUM") as sc_psum,
            tc.tile_pool(name="yps_psum", bufs=2, space="PSUM") as yps_psum,
            tc.tile_pool(name="sum_psum", bufs=1, space="PSUM") as sum_psum,
            tc.tile_pool(name="o_psum", bufs=1, space="PSUM") as o_psum,
        ):
            def oproj_window(tw):
                # out[:, 512tw:512tw+512] = sum_h woT_h^T @ yT_h window
                for ep in range(4):  # pairs of 128-col e chunks per out DMA
                    ot = o_out.tile([128, 2, 512], F16, name="ot")
                    for sub in range(2):
                        ec = 2 * ep + sub
                        ops = o_psum.tile([128, 512], F32, name="ops")
                        for h in range(HPC):
                            nc.tensor.matmul(
                                ops,
                                wo_sb[:, h, ec * 128:(ec + 1) * 128],
                                yT[h][:, tw * 512:(tw + 1) * 512],
                                start=(h == 0), stop=(h == HPC - 1),
                                skip_group_check=True,
                            )
                        if ec % 2 == 0:
                            nc.scalar.copy(out=ot[:, sub, :], in_=ops)
                        else:
                            nc.vector.tensor_copy(out=ot[:, sub, :], in_=ops)
                    nc.sync.dma_start(
                        out=outT.rearrange("(a p) t -> p a t", p=128)
                            [:, 2 * ep:2 * ep + 2,
                             tw * 512:(tw + 1) * 512],
                        in_=ot,
                    )

            # per q-tile, k chunks qt..qt+4 of kT; slot 0 takes the strict
            # upper mask, slot 4 (diag) the lower-incl mask.
            def issue_scores(qt, h):
                sc = sc_psum.tile([128, 5, 128], F32, name="sc", tag="sc")
                qs = qT[h][:, qt * 128:(qt + 1) * 128]
                for j in range(5):
                    nc.tensor.matmul(
                        sc[:, j, :],
                        kT[h][:, (qt + j) * 128:(qt + j + 1) * 128],
                        qs, start=True, stop=True, skip_group_check=True,
                    )
                return sc

            def consume_front(qt, h, sc):
                pm = pm_pool.tile([128, 5, 128], F16)
                nc.scalar.activation(pm, sc, AF.Exp,
                                     scale=ATTN_SCALE, bias=ebias_sb)
                nc.vector.tensor_tensor(out=pm[:, 0, :], in0=pm[:, 0, :],
                                        in1=masks[:, 0, :], op=AOP.mult)
                nc.vector.tensor_tensor(out=pm[:, 4, :], in0=pm[:, 4, :],
                                        in1=masks[:, 1, :], op=AOP.mult)
                return pm

            def consume_back(qt, h, sc, pm):
                yps = yps_psum.tile([128, 128], F32, name="yps")
                sums = sum_psum.tile([1, 128], F32, name="sums")
                for i, j in enumerate((1, 2, 3, 0, 4)):  # unmasked slots first
                    nc.tensor.matmul(
                        sums, ones_col, pm[:, j, :],
                        start=(i == 0), stop=(i == 4), skip_group_check=True,
                    )
                    nc.tensor.matmul(
                        yps, vbf[h][:, qt + j, :], pm[:, j, :],
                        start=(i == 0), stop=(i == 4), skip_group_check=True,
                    )
                with tc.high_priority(offset=40):
                    sums2 = smallB.tile([1, 128], F32)
                    nc.vector.tensor_sub(sums2, sums,
                                         pad2[:, qt * 128:(qt + 1) * 128])
                    recip = smallB.tile([1, 128], F32)
                    nc.vector.reciprocal(recip, sums2)
                    # broadcast 1/sum across partitions on the Pool engine
                    bc_sb = smallB.tile([128, 128], F32, name="bc_sb")
                    nc.gpsimd.partition_broadcast(bc_sb, recip)
                nc.vector.tensor_tensor(
                    out=yT[h][:, qt * 128:(qt + 1) * 128],
                    in0=yps, in1=bc_sb, op=AOP.mult)

            # 2-deep software pipeline: scores for steps i+1,i+2 are already
            # issued before the exp-dependent matmuls of step i, so the PE
            # stays continuously busy (and at full p-state).
            steps = [(qt, h) for qt in range(NQT) for h in range(HPC)]
            tiles = [issue_scores(*steps[0]), issue_scores(*steps[1])]
            pms = [consume_front(*steps[0], tiles[0])]
            for i, (qt, h) in enumerate(steps):
                if i + 2 < len(steps):
                    tiles.append(issue_scores(*steps[i + 2]))
                if i + 1 < len(steps):
                    pms.append(consume_front(*steps[i + 1], tiles[i + 1]))
                consume_back(qt, h, tiles[i], pms[i])
                tiles[i] = pms[i] = None  # release references
                if h == HPC - 1 and qt % 4 == 3:
                    oproj_window(qt // 4)


_NC_CACHE = None


def _get_nc():
    global _NC_CACHE
    if _NC_CACHE is None:
        _NC_CACHE = build_kernel()
    return _NC_CACHE


# permutation putting rope-rotated dim pairs adjacent: [x1, x2, rest]
ROPE_PERM = np.concatenate([
    np.arange(0, 32), np.arange(64, 96), np.arange(32, 64), np.arange(96, 128)
])


def _rope_tables(positions):
    keep = HD // 4
    active = (1.0 / ROPE_BASE) ** np.linspace(0.0, 1.0, keep, dtype=np.float32)
    theta = positions[:, None].astype(np.float32) * active[None, :]  # [n, 32]
    return np.cos(theta).astype(np.float32), np.sin(theta).astype(np.float32)


def make_in_maps(x, ve, lambdas, qkvo_w):
    """Build the 8 per-core input maps from full inputs (host-side sharding)."""
    x2 = x.reshape(T, DIM)
    ve2 = ve.reshape(T, DIM)
    qw, kw, vw, ow = qkvo_w[0], qkvo_w[1], qkvo_w[2], qkvo_w[3]
    l0, l1 = float(lambdas[0]), float(lambdas[1])

    in_maps = []
    for c in range(8):
        s, g = divmod(c, G)
        h0, h1 = HPC * g, HPC * g + 1
        lo = TC * s - WINDOW  # first k/v row (may be negative -> zero pad)
        hi = TC * s + TC

        # xT slice with zero pad
        xs = np.zeros((TK, DIM), np.float32)
        src_lo = max(lo, 0)
        xs[src_lo - lo:, :] = x2[src_lo:hi, :]
        xTc = np.ascontiguousarray(xs.T).astype(np.float16)

        # fused qkv weight, transposed: cols = q0 q1 k0 k1 v0 v1;
        # q/k head dims permuted so rope pairs are adjacent
        wcols = []
        for wmat, perm in ((qw, ROPE_PERM), (kw, ROPE_PERM), (vw, None)):
            for h in (h0, h1):
                blk = wmat[h * HD:(h + 1) * HD, :]
                if perm is not None:
                    blk = blk[perm, :]
                wcols.append(blk.T)
        wqkvT = np.ascontiguousarray(
            np.concatenate(wcols, axis=1)).astype(np.float16)

        woT = np.ascontiguousarray(
            ow[:, h0 * HD:(h1 + 1) * HD].T).astype(np.float16)

        # ve pre-scaled by lambda1 (folds the lam1 multiply into the DMA)
        ves = np.zeros((TK, HPC * HD), np.float32)
        ves[src_lo - lo:, :] = l1 * ve2[src_lo:hi, h0 * HD:(h1 + 1) * HD]
        ves = ves.astype(np.float16)

        pos = np.clip(np.arange(lo, hi), 0, None)
        cosv, sinv = _rope_tables(pos)
        rope_tbl = np.concatenate(
            [cosv, cosv, sinv, -sinv], axis=1).astype(np.float16)

        # Rsqrt scale/bias folding lam0 into the v norm:
        # rs_v = lam0/sqrt(mean+eps) = rsqrt(ssum/(HD*lam0^2) + eps/lam0^2)
        lam_row = np.array([1.0 / (HD * l0 * l0), EPS / (l0 * l0)], np.float32)
        lam = np.tile(lam_row.reshape(1, 2), (128, 1)).astype(np.float32)

        # count of padded (zero) keys in each row's window; the device scales
        # this by its own exp(EXP_BIAS) value before subtracting from the
        # softmax denominator
        pc = np.zeros(TC, np.float32)
        if s == 0:
            i = np.arange(TC)
            pc = np.maximum(0.0, WINDOW - 1.0 - i).astype(np.float32)

        in_maps.append({
            "xT": xTc, "wqkvT": wqkvT, "woT": woT, "ve": ves,
            "rope_tbl": rope_tbl, "lam": lam, "padcnt": pc,
        })
    return in_maps


def gather_out(results):
    """Sum per-core partial outputs into the full [1, T, DIM] array."""
    outT_full = np.zeros((DIM, T), np.float32)
    for c in range(8):
        s = c // G
        outT_full[:, TC * s:TC * (s + 1)] += np.asarray(
            results[c]["outT"], np.float32)
    return np.ascontiguousarray(outT_full.T).reshape(1, T, DIM)


def kernel(x, ve, lambdas, qkvo_w, window):
    assert int(window) == WINDOW
    from concourse.bass_utils import run_bass_kernel_spmd

    x = np.asarray(x, np.float32)
    ve = np.asarray(ve, np.float32)
    lambdas = np.asarray(lambdas, np.float32)
    qkvo_w = np.asarray(qkvo_w, np.float32)

    nc = _get_nc()
    in_maps = make_in_maps(x, ve, lambdas, qkvo_w)
    res = run_bass_kernel_spmd(nc, in_maps, core_ids=list(range(8)))
    return gather_out(res.results)


if __name__ == "__main__":
    nc = _get_nc()
    print("kernel built ok")


# revision 41
# speedup vs baseline: 1.0347x; 1.0347x over previous
"""Trainium2 Bass kernel for sliding-window causal self-attention.

Reference computation (B=1, T=4096, H=8 heads, head_dim=128, DIM=1024):
  qkv = x @ w_qkv.T; q,k = rms_norm -> rope; v = lam0*rms_norm(v) + lam1*ve
  scores = (q k^T) * 0.12 with sliding-window causal mask (0 <= i-j < 512)
  y = softmax(scores) @ v;  out = y @ o_w.T

Sharding over 8 cores: 2 sequence halves (S) x 4 head-pair groups (G).
Core c = 4*s + g handles t in [2048s, 2048(s+1)) for heads {2g, 2g+1}.
Each core reads its x rows plus a 512-row halo of preceding rows (for k/v),
computes its partial output projection over its 2 heads, and the host sums
the 4 partials per half and concatenates the halves. No on-chip collectives.

Attention uses a transposeless [kj, qi] scores layout: q-tiles are processed
in pairs (256 queries, 768-key window, 6 key chunks) so every matmul has a
moving free dim >= 256, which is required for full-rate float32r matmuls.
Softmax runs without max-subtraction (scores are bounded by 0.12*128), the
kj-sum is done on the PE with a ones vector, and the reciprocal is broadcast
across partitions with a rank-1 matmul. The output projection is interleaved
into the attention loop so its DMA overlaps compute. Elementwise work is
spread across DVE / ScalarE / GpSimd to keep all engines busy.
"""

import sys

sys.path.insert(0, "/opt/trn_rl_repo")

import numpy as np

import concourse.bass as bass
import concourse.mybir as mybir
import concourse.tile as tile
from concourse import bacc
from concourse.bass_utils import run_bass_kernel_spmd
from concourse.masks import make_identity

# Problem constants
T = 4096
DIM = 1024
H = 8
HD = 128
WINDOW = 512
ATTN_SCALE = 0.12
ROPE_BASE = 1024.0
EPS = 1e-6

# Sharding
S = 2          # sequence halves
G = 4          # head groups (2 heads each)
HPC = H // G   # heads per core = 2
TC = T // S    # own rows per core = 2048
TK = TC + WINDOW  # k/v rows incl. halo = 2560
NQT = TC // 128   # q tiles per head = 16
NKC = TK // 128   # k chunks = 20
NPR = TC // 256   # q pairs per head = 8
PW = 256 + WINDOW  # pair window = 768
NPC = PW // 128    # chunks per pair window = 6
EW = 3 * HPC * HD  # fused qkv width per core = 768

F32 = mybir.dt.float32
F32R = mybir.dt.float32r

AOP = mybir.AluOpType
AF = mybir.ActivationFunctionType


def build_kernel():
    nc = bacc.Bacc()

    # Per-core DRAM I/O (shapes identical across cores; data differs).
    xT = nc.declare_dram_parameter("xT", [DIM, TK], F32, isOutput=False)
    wqkvT = nc.declare_dram_parameter("wqkvT", [DIM, EW], F32, isOutput=False)
    woT = nc.declare_dram_parameter("woT", [HPC * HD, DIM], F32, isOutput=False)
    ve = nc.declare_dram_parameter("ve", [TK, HPC * HD], F32, isOutput=False)
    cosT = nc.declare_dram_parameter("cosT", [TK, 32], F32, isOutput=False)
    sinT = nc.declare_dram_parameter("sinT", [TK, 32], F32, isOutput=False)
    lam = nc.declare_dram_parameter("lam", [128, 4], F32, isOutput=False)
    padcnt = nc.declare_dram_parameter("padcnt", [TC], F32, isOutput=False)
    outT = nc.declare_dram_parameter("outT", [DIM, TC], F32, isOutput=True)

    with tile.TileContext(nc) as tc:
        _trace_body(nc, tc, xT, wqkvT, woT, ve, cosT, sinT, lam, padcnt, outT)

    nc.compile()
    return nc


def _trace_body(nc, tc, xT, wqkvT, woT, ve, cosT, sinT, lam, padcnt, outT):
    import contextlib

    ctx = contextlib.ExitStack()
    with ctx:
        const = ctx.enter_context(tc.tile_pool(name="const", bufs=1))
        persist = ctx.enter_context(tc.tile_pool(name="persist", bufs=1))

        # ---- constants needed by phase A (w split per d-chunk so the
        # first projection matmuls can start early) ----
        w_sb = const.tile([128, 8, EW], F32R)  # wqkvT as [dpart, dchunk, e]
        wq_r = wqkvT.rearrange("(a p) e -> p a e", p=128).bitcast(F32R)
        for dch in range(4):
            nc.sync.dma_start(out=w_sb[:, dch, :], in_=wq_r[:, dch, :])
        cos_sb = const.tile([128, NKC, 32], F32)
        sin_sb = const.tile([128, NKC, 32], F32)
        lam_sb = const.tile([128, 4], F32)

        identity = const.tile([128, 128], F32R)
        idf = const.tile([128, 128], F32)
        make_identity(nc, idf)
        nc.vector.tensor_copy(out=identity, in_=idf)

        eps_sb = const.tile([128, 1], F32)
        nc.vector.memset(eps_sb, EPS)

        # ---- B/C constant tiles (DMAs deferred until after phase A) ----
        wo_sb = const.tile([128, HPC, DIM], F32R)  # woT as [ddpart, head, e]
        pad_r = const.tile([1, TC], F32)

        onescf = const.tile([128, 1], F32)
        nc.vector.memset(onescf, 1.0)
        ones_col = const.tile([128, 1], F32R)
        nc.vector.tensor_copy(out=ones_col, in_=onescf)

        # Band masks in [kj, ci, qi] orientation for pair-window chunks.
        # Chunk c of a pair window is valid iff qi+1 <= 128c + kj <= qi+512.
        # Chunks 2,3 are always fully valid; 0,1 need the lower bound and
        # 4,5 the upper bound.
        maskA = const.tile([128, 2, 256], F32)  # chunks 0,1
        nc.gpsimd.memset(maskA, 1.0)
        nc.gpsimd.affine_select(
            out=maskA, in_=maskA, compare_op=AOP.is_ge, fill=0.0,
            base=-1, channel_multiplier=1, pattern=[[128, 2], [-1, 256]],
        )
        maskB = const.tile([128, 2, 256], F32)  # chunks 4,5
        nc.gpsimd.memset(maskB, 1.0)
        nc.gpsimd.affine_select(
            out=maskB, in_=maskB, compare_op=AOP.is_ge, fill=0.0,
            base=0, channel_multiplier=-1, pattern=[[-128, 2], [1, 256]],
        )

        # ---- persistent activations ----
        # qT/kT: [dd, t] per head; v: [t(kj) part, chunk, dd]; yT: [dd, t].
        qT = [persist.tile([128, TC], F32R, name=f"qT{h}") for h in range(HPC)]
        kT = [persist.tile([128, TK], F32R, name=f"kT{h}") for h in range(HPC)]
        vbf = [persist.tile([128, NKC, HD], F32R, name=f"vbf{h}") for h in range(HPC)]
        yT = [persist.tile([128, TC], F32R, name=f"yT{h}") for h in range(HPC)]

        # ================= Phase A: QKV projection + norm/rope =================
        with (
            tc.tile_pool(name="xt_pool", bufs=2) as xt_pool,
            tc.tile_pool(name="ve_pool", bufs=2) as ve_pool,
            tc.tile_pool(name="stageA", bufs=4) as stageA,
            tc.tile_pool(name="smallA", bufs=8) as smallA,
            tc.tile_pool(name="proj_psum", bufs=3, space="PSUM") as proj_psum,
            tc.tile_pool(name="tp_psum", bufs=2, space="PSUM") as tp_psum,
        ):
            xT_r = xT.rearrange("(a p) t -> p a t", p=128)  # [128, 8, TK]
            ve_r = ve.rearrange("(a p) d -> p a d", p=128)  # [128, 20, 256]
            TB = 512  # t rows per x block load
            for tb in range(TK // TB):
                xt = xt_pool.tile([128, 8, TB], F32R)
                # split into two DMAs so compute can start on the first half
                xsrc = xT_r[:, :, tb * TB:(tb + 1) * TB].bitcast(F32R)
                nc.sync.dma_start(out=xt[:, 0:4, :], in_=xsrc[:, 0:4, :])
                nc.sync.dma_start(out=xt[:, 4:8, :], in_=xsrc[:, 4:8, :])
                vet = ve_pool.tile([128, 4, HPC * HD], F32)
                nc.sync.dma_start(out=vet, in_=ve_r[:, tb * 4:(tb + 1) * 4, :])
                if tb == 0:
                    # bulk loads deferred behind the first x block
                    for dch in range(4, 8):
                        nc.sync.dma_start(out=w_sb[:, dch, :], in_=wq_r[:, dch, :])
                    nc.sync.dma_start(
                        out=cos_sb, in_=cosT.rearrange("(a p) f -> p a f", p=128))
                    nc.sync.dma_start(
                        out=sin_sb, in_=sinT.rearrange("(a p) f -> p a f", p=128))
                    nc.sync.dma_start(out=lam_sb, in_=lam[:])
                for tsub in range(TB // 128):
                    c = tb * (TB // 128) + tsub  # t-chunk index, 0..19
                    psum = proj_psum.tile([128, EW], F32)
                    for dch in range(8):
                        lhsT = xt[:, dch, tsub * 128:(tsub + 1) * 128]
                        if c >= 4:
                            nc.tensor.matmul(
                                psum[:, 0:512], lhsT, w_sb[:, dch, 0:512],
                                start=(dch == 0), stop=(dch == 7),
                            )
                        else:  # halo rows need only k,v
                            nc.tensor.matmul(
                                psum[:, 256:512], lhsT, w_sb[:, dch, 256:512],
                                start=(dch == 0), stop=(dch == 7),
                            )
                        nc.tensor.matmul(
                            psum[:, 512:EW], lhsT, w_sb[:, dch, 512:EW],
                            start=(dch == 0), stop=(dch == 7),
                        )
                    # psum segments: q0 q1 k0 k1 v0 v1, each [128, 128]
                    psum6 = psum.rearrange("p (s d) -> p s d", s=6)

                    # RMS-norm scales (halo chunks skip the q segments).
                    # Square on ACT with fused per-segment row-sum accumulation.
                    s0 = 0 if c >= 4 else 2
                    sq = stageA.tile([128, 6, HD], F32)
                    ssum = smallA.tile([128, 6], F32)
                    for sg in range(s0, 6):
                        nc.scalar.activation(
                            sq[:, sg, :], psum6[:, sg, :], AF.Square,
                            accum_out=ssum[:, sg:sg + 1],
                        )
                    # rms for q,k (eps bias) and v (lam0 folded via scale/bias)
                    rms = smallA.tile([128, 6], F32)
                    nc.scalar.activation(rms[:, s0:4], ssum[:, s0:4], AF.Sqrt,
                                         bias=eps_sb, scale=1.0 / HD)
                    nc.scalar.activation(rms[:, 4:6], ssum[:, 4:6], AF.Sqrt,
                                         bias=lam_sb[:, 3:4],
                                         scale=lam_sb[:, 2:3])
                    rs = smallA.tile([128, 6], F32)
                    nc.vector.reciprocal(rs[:, s0:6], rms[:, s0:6])

                    # normalize segments in one DVE op -> staging (f32r)
                    st6 = stageA.tile([128, 6, HD], F32R)
                    nc.vector.tensor_tensor(
                        out=st6[:, s0:6, :], in0=psum6[:, s0:6, :],
                        in1=rs[:, s0:6, None].to_broadcast([128, 6 - s0, HD]),
                        op=AOP.mult,
                    )
                    st6f = st6.bitcast(F32)

                    # v = lam1 * ve + v_normed (gpsimd; all-SBUF).
                    # Pool has no TensorScalarPtr, so use two tensor_tensor
                    # ops with a broadcast lam1 operand.
                    vel = stageA.tile([128, 2, HD], F32, name="vel")
                    nc.gpsimd.tensor_tensor(
                        out=vel, in0=vet[:, tsub, :].rearrange("p (h d) -> p h d", h=2),
                        in1=lam_sb[:, 1:2, None].to_broadcast([128, 2, HD]),
                        op=AOP.mult,
                    )
                    for h in range(HPC):
                        nc.vector.tensor_tensor(
                            out=vbf[h][:, c, :], in0=vel[:, h, :],
                            in1=st6f[:, 4 + h, :], op=AOP.add,
                        )

                    # rope on q,k (dims 0:32 rotate with dims 64:96); gpsimd
                    nseg = 4 - s0
                    cs = cos_sb[:, c:c + 1, :].to_broadcast([128, nseg, 32])
                    sn = sin_sb[:, c:c + 1, :].to_broadcast([128, nseg, 32])
                    x1 = st6f[:, s0:4, 0:32]
                    x2 = st6f[:, s0:4, 64:96]
                    t1 = stageA.tile([128, 4, 32], F32)
                    t2 = stageA.tile([128, 4, 32], F32)
                    t3 = stageA.tile([128, 4, 32], F32)
                    t4 = stageA.tile([128, 4, 32], F32)
                    nc.vector.tensor_tensor(out=t1[:, s0:4, :], in0=x1, in1=cs, op=AOP.mult)
                    nc.vector.tensor_tensor(out=t2[:, s0:4, :], in0=x2, in1=sn, op=AOP.mult)
                    nc.gpsimd.tensor_tensor(out=t3[:, s0:4, :], in0=x1, in1=sn, op=AOP.mult)
                    nc.gpsimd.tensor_tensor(out=t4[:, s0:4, :], in0=x2, in1=cs, op=AOP.mult)
                    nc.vector.tensor_add(st6[:, s0:4, 0:32], t1[:, s0:4, :], t2[:, s0:4, :])
                    nc.vector.tensor_sub(st6[:, s0:4, 64:96], t4[:, s0:4, :], t3[:, s0:4, :])

                    # transpose q,k into [dd, t] persistent buffers (f32r)
                    for h in range(HPC):
                        if c >= 4:  # q exists only for own rows
                            tq = tp_psum.tile([128, 128], F32R, name="tq", tag="tp")
                            nc.tensor.transpose(tq, st6[:, h, :], identity)
                            nc.vector.tensor_copy(
                                out=qT[h][:, (c - 4) * 128:(c - 3) * 128], in_=tq)
                        tk = tp_psum.tile([128, 128], F32R, name="tk", tag="tp")
                        nc.tensor.transpose(tk, st6[:, 2 + h, :], identity)
                        nc.vector.tensor_copy(out=kT[h][:, c * 128:(c + 1) * 128],
                                              in_=tk)

        nc.sync.dma_start(
            out=wo_sb, in_=woT.rearrange("(a p) e -> p a e", p=128).bitcast(F32R))
        nc.sync.dma_start(out=pad_r, in_=padcnt.rearrange("(a t) -> a t", a=1))

        # ====== Phase B+C: banded attention with interleaved out-projection ===
        with (
            tc.tile_pool(name="pm_pool", bufs=3) as pm_pool,
            tc.tile_pool(name="smallB", bufs=8) as smallB,
            tc.tile_pool(name="o_out", bufs=4) as o_out,
            tc.tile_pool(name="sc_psum", bufs=3, space="PSUM") as sc_psum,
            tc.tile_pool(name="sum_psum", bufs=1, space="PSUM") as sum_psum,
            tc.tile_pool(name="y_psum", bufs=1, space="PSUM") as y_psum,
            tc.tile_pool(name="o_psum", bufs=2, space="PSUM") as o_psum,
        ):
            def oproj_window(tw):
                # out[:, 512tw:512tw+512] = sum_h woT_h^T @ yT_h window
                for ec in range(8):
                    ops = o_psum.tile([128, 512], F32, name="ops")
                    for h in range(HPC):
                        nc.tensor.matmul(
                            ops,
                            wo_sb[:, h, ec * 128:(ec + 1) * 128],
                            yT[h][:, tw * 512:(tw + 1) * 512],
                            start=(h == 0), stop=(h == HPC - 1),
                            skip_group_check=True,
                        )
                    ot = o_out.tile([128, 512], F32, name="ot")
                    if ec % 2 == 0:
                        nc.scalar.copy(out=ot, in_=ops)
                    else:
                        nc.vector.tensor_copy(out=ot, in_=ops)
                    nc.sync.dma_start(
                        out=outT[ec * 128:(ec + 1) * 128,
                                 tw * 512:(tw + 1) * 512],
                        in_=ot,
                    )

            for pr in range(NPR):
                for h in range(HPC):
                    qs = qT[h][:, pr * 256:(pr + 1) * 256]
                    pm = pm_pool.tile([128, NPC, 256], F32R)
                    sums = sum_psum.tile([1, 256], F32, name="sums")
                    yps = y_psum.tile([128, 256], F32, name="yps")
                    # masked chunk pairs first so the final accumulation
                    # tail has no Pool mask op on its critical path
                    for i, wp in enumerate((0, 2, 1)):  # chunk pairs
                        sc = sc_psum.tile([128, 2, 256], F32, name="sc", tag="sc")
                        for j in range(2):
                            wc = 2 * wp + j
                            nc.tensor.matmul(
                                sc[:, j, :],
                                kT[h][:, (2 * pr + wc) * 128:(2 * pr + wc + 1) * 128],
                                qs, start=True, stop=True, skip_group_check=True,
                            )
                        nc.scalar.activation(pm[:, 2 * wp:2 * wp + 2, :], sc,
                                             AF.Exp, scale=ATTN_SCALE)
                        if wp == 0:
                            nc.vector.tensor_tensor(
                                out=pm[:, 0:2, :], in0=pm[:, 0:2, :].bitcast(F32),
                                in1=maskA, op=AOP.mult)
                        elif wp == 2:
                            nc.vector.tensor_tensor(
                                out=pm[:, 4:6, :], in0=pm[:, 4:6, :].bitcast(F32),
                                in1=maskB, op=AOP.mult)
                        for j in range(2):
                            wc = 2 * wp + j
                            nc.tensor.matmul(
                                sums, ones_col, pm[:, wc, :],
                                start=(i == 0 and j == 0),
                                stop=(i == 2 and j == 1),
                                skip_group_check=True,
                            )
                            nc.tensor.matmul(
                                yps, vbf[h][:, 2 * pr + wc, :], pm[:, wc, :],
                                start=(i == 0 and j == 0),
                                stop=(i == 2 and j == 1),
                                skip_group_check=True,
                            )
                    with tc.high_priority(offset=40):
                        sums2 = smallB.tile([1, 256], F32)
                        nc.vector.tensor_sub(sums2, sums,
                                             pad_r[:, pr * 256:(pr + 1) * 256])
                        recip = smallB.tile([1, 256], F32)
                        nc.vector.reciprocal(recip, sums2)
                        # broadcast 1/sum across partitions on the Pool engine
                        bc_sb = smallB.tile([128, 256], F32, name="bc_sb")
                        nc.gpsimd.partition_broadcast(bc_sb, recip)
                    # evacuate with the 1/sum normalization fused (cast f32r)
                    nc.vector.tensor_tensor(
                        out=yT[h][:, pr * 256:(pr + 1) * 256],
                        in0=yps, in1=bc_sb, op=AOP.mult)
                if pr % 2 == 1:
                    oproj_window(pr // 2)


_NC_CACHE = None


def _get_nc():
    global _NC_CACHE
    if _NC_CACHE is None:
        _NC_CACHE = build_kernel()
    return _NC_CACHE


def _rope_tables(positions):
    keep = HD // 4
    active = (1.0 / ROPE_BASE) ** np.linspace(0.0, 1.0, keep, dtype=np.float32)
    theta = positions[:, None].astype(np.float32) * active[None, :]  # [n, 32]
    return np.cos(theta).astype(np.float32), np.sin(theta).astype(np.float32)


def make_in_maps(x, ve, lambdas, qkvo_w):
    """Build the 8 per-core input maps from full inputs (host-side sharding)."""
    x2 = x.reshape(T, DIM)
    ve2 = ve.reshape(T, DIM)
    qw, kw, vw, ow = qkvo_w[0], qkvo_w[1], qkvo_w[2], qkvo_w[3]

    in_maps = []
    for c in range(8):
        s, g = divmod(c, G)
        h0, h1 = HPC * g, HPC * g + 1
        lo = TC * s - WINDOW  # first k/v row (may be negative -> zero pad)
        hi = TC * s + TC

        # xT slice with zero pad
        xs = np.zeros((TK, DIM), np.float32)
        src_lo = max(lo, 0)
        xs[src_lo - lo:, :] = x2[src_lo:hi, :]
        xTc = np.ascontiguousarray(xs.T)

        # fused qkv weight, transposed: cols = q0 q1 k0 k1 v0 v1
        wcols = []
        for wmat in (qw, kw, vw):
            for h in (h0, h1):
                wcols.append(wmat[h * HD:(h + 1) * HD, :].T)
        wqkvT = np.ascontiguousarray(np.concatenate(wcols, axis=1))

        woT = np.ascontiguousarray(ow[:, h0 * HD:(h1 + 1) * HD].T)

        ves = np.zeros((TK, HPC * HD), np.float32)
        ves[src_lo - lo:, :] = ve2[src_lo:hi, h0 * HD:(h1 + 1) * HD]

        pos = np.clip(np.arange(lo, hi), 0, None)
        cosT, sinT = _rope_tables(pos)

        l0, l1 = float(lambdas[0]), float(lambdas[1])
        lam_row = np.array([l0, l1, 1.0 / (HD * l0 * l0), EPS / (l0 * l0)],
                           np.float32)
        lam = np.tile(lam_row.reshape(1, 4), (128, 1)).astype(np.float32)

        pc = np.zeros(TC, np.float32)
        if s == 0:
            i = np.arange(TC)
            pc = np.maximum(0.0, WINDOW - 1.0 - i).astype(np.float32)

        in_maps.append({
            "xT": xTc, "wqkvT": wqkvT, "woT": woT, "ve": ves,
            "cosT": cosT, "sinT": sinT, "lam": lam, "padcnt": pc,
        })
    return in_maps


def kernel(x, ve, lambdas, qkvo_w, window):
    assert int(window) == WINDOW
    x = np.asarray(x, np.float32)
    ve = np.asarray(ve, np.float32)
    lambdas = np.asarray(lambdas, np.float32)
    qkvo_w = np.asarray(qkvo_w, np.float32)

    nc = _get_nc()
    in_maps = make_in_maps(x, ve, lambdas, qkvo_w)
    res = run_bass_kernel_spmd(nc, in_maps, core_ids=list(range(8)))

    outT_full = np.zeros((DIM, T), np.float32)
    for c in range(8):
        s = c // G
        outT_full[:, TC * s:TC * (s + 1)] += res.results[c]["outT"]
    return np.ascontiguousarray(outT_full.T).reshape(1, T, DIM)


if __name__ == "__main__":
    nc = _get_nc()
    print("kernel built ok")


# revision 44
# speedup vs baseline: 1.0521x; 1.0168x over previous
"""Trainium2 Bass kernel for sliding-window causal self-attention.

Reference computation (B=1, T=4096, H=8 heads, head_dim=128, DIM=1024):
  qkv = x @ w_qkv.T; q,k = rms_norm -> rope; v = lam0*rms_norm(v) + lam1*ve
  scores = (q k^T) * 0.12 with sliding-window causal mask (0 <= i-j < 512)
  y = softmax(scores) @ v;  out = y @ o_w.T

Sharding over 8 cores: 2 sequence halves (S) x 4 head-pair groups (G).
Core c = 4*s + g handles t in [2048s, 2048(s+1)) for heads {2g, 2g+1}.
Each core reads its x rows plus a 512-row halo of preceding rows (for k/v),
computes its partial output projection over its 2 heads, and the host sums
the 4 partials per half and concatenates the halves. No on-chip collectives.

Attention uses a transposeless [kj, qi] scores layout: q-tiles are processed
in pairs (256 queries, 768-key window, 6 key chunks) so every matmul has a
moving free dim >= 256, which is required for full-rate float32r matmuls.
Softmax runs without max-subtraction (scores are bounded by 0.12*128), the
kj-sum is done on the PE with a ones vector, and the reciprocal is broadcast
across partitions with a rank-1 matmul. The output projection is interleaved
into the attention loop so its DMA overlaps compute. Elementwise work is
spread across DVE / ScalarE / GpSimd to keep all engines busy.
"""

import sys

sys.path.insert(0, "/opt/trn_rl_repo")

import numpy as np

import concourse.bass as bass
import concourse.mybir as mybir
import concourse.tile as tile
from concourse import bacc
from concourse.bass_utils import run_bass_kernel_spmd
from concourse.masks import make_identity

# Problem constants
T = 4096
DIM = 1024
H = 8
HD = 128
WINDOW = 512
ATTN_SCALE = 0.12
ROPE_BASE = 1024.0
EPS = 1e-6

# Sharding
S = 2          # sequence halves
G = 4          # head groups (2 heads each)
HPC = H // G   # heads per core = 2
TC = T // S    # own rows per core = 2048
TK = TC + WINDOW  # k/v rows incl. halo = 2560
NQT = TC // 128   # q tiles per head = 16
NKC = TK // 128   # k chunks = 20
NPR = TC // 256   # q pairs per head = 8
PW = 256 + WINDOW  # pair window = 768
NPC = PW // 128    # chunks per pair window = 6
EW = 3 * HPC * HD  # fused qkv width per core = 768

F32 = mybir.dt.float32
F32R = mybir.dt.float32r

AOP = mybir.AluOpType
AF = mybir.ActivationFunctionType


def build_kernel():
    nc = bacc.Bacc()

    # Per-core DRAM I/O (shapes identical across cores; data differs).
    xT = nc.declare_dram_parameter("xT", [DIM, TK], F32, isOutput=False)
    wqkvT = nc.declare_dram_parameter("wqkvT", [DIM, EW], F32, isOutput=False)
    woT = nc.declare_dram_parameter("woT", [HPC * HD, DIM], F32, isOutput=False)
    ve = nc.declare_dram_parameter("ve", [TK, HPC * HD], F32, isOutput=False)
    cosT = nc.declare_dram_parameter("cosT", [TK, 32], F32, isOutput=False)
    sinT = nc.declare_dram_parameter("sinT", [TK, 32], F32, isOutput=False)
    lam = nc.declare_dram_parameter("lam", [128, 4], F32, isOutput=False)
    padcnt = nc.declare_dram_parameter("padcnt", [TC], F32, isOutput=False)
    outT = nc.declare_dram_parameter("outT", [DIM, TC], F32, isOutput=True)

    with tile.TileContext(nc) as tc:
        _trace_body(nc, tc, xT, wqkvT, woT, ve, cosT, sinT, lam, padcnt, outT)

    nc.compile()
    return nc


def _trace_body(nc, tc, xT, wqkvT, woT, ve, cosT, sinT, lam, padcnt, outT):
    import contextlib

    ctx = contextlib.ExitStack()
    with ctx:
        const = ctx.enter_context(tc.tile_pool(name="const", bufs=1))
        persist = ctx.enter_context(tc.tile_pool(name="persist", bufs=1))

        # ---- constants needed by phase A (w split per d-chunk so the
        # first projection matmuls can start early) ----
        w_sb = const.tile([128, 8, EW], F32R)  # wqkvT as [dpart, dchunk, e]
        wq_r = wqkvT.rearrange("(a p) e -> p a e", p=128).bitcast(F32R)
        cos_sb = const.tile([128, NKC, 32], F32)
        sin_sb = const.tile([128, NKC, 32], F32)
        lam_sb = const.tile([128, 4], F32)

        identity = const.tile([128, 128], F32R)
        idf = const.tile([128, 128], F32)
        make_identity(nc, idf)
        nc.vector.tensor_copy(out=identity, in_=idf)

        eps_sb = const.tile([128, 1], F32)
        nc.vector.memset(eps_sb, EPS)

        # ---- B/C constant tiles (DMAs deferred until after phase A) ----
        wo_sb = const.tile([128, HPC, DIM], F32R)  # woT as [ddpart, head, e]
        pad_r = const.tile([1, TC], F32)

        onescf = const.tile([128, 1], F32)
        nc.vector.memset(onescf, 1.0)
        ones_col = const.tile([128, 1], F32R)
        nc.vector.tensor_copy(out=ones_col, in_=onescf)

        # Band masks in [kj, ci, qi] orientation for pair-window chunks.
        # Chunk c of a pair window is valid iff qi+1 <= 128c + kj <= qi+512.
        # Chunks 2,3 are always fully valid; 0,1 need the lower bound and
        # 4,5 the upper bound.
        maskA = const.tile([128, 2, 256], F32)  # chunks 0,1
        nc.gpsimd.memset(maskA, 1.0)
        nc.gpsimd.affine_select(
            out=maskA, in_=maskA, compare_op=AOP.is_ge, fill=0.0,
            base=-1, channel_multiplier=1, pattern=[[128, 2], [-1, 256]],
        )
        maskB = const.tile([128, 2, 256], F32)  # chunks 4,5
        nc.gpsimd.memset(maskB, 1.0)
        nc.gpsimd.affine_select(
            out=maskB, in_=maskB, compare_op=AOP.is_ge, fill=0.0,
            base=0, channel_multiplier=-1, pattern=[[-128, 2], [1, 256]],
        )

        # ---- persistent activations ----
        # qT/kT: [dd, t] per head; v: [t(kj) part, chunk, dd]; yT: [dd, t].
        qT = [persist.tile([128, TC], F32R, name=f"qT{h}") for h in range(HPC)]
        kT = [persist.tile([128, TK], F32R, name=f"kT{h}") for h in range(HPC)]
        vbf = [persist.tile([128, NKC, HD], F32R, name=f"vbf{h}") for h in range(HPC)]
        yT = [persist.tile([128, TC], F32R, name=f"yT{h}") for h in range(HPC)]

        # ================= Phase A: QKV projection + norm/rope =================
        with (
            tc.tile_pool(name="xt_pool", bufs=2) as xt_pool,
            tc.tile_pool(name="ve_pool", bufs=2) as ve_pool,
            tc.tile_pool(name="stageA", bufs=4) as stageA,
            tc.tile_pool(name="smallA", bufs=8) as smallA,
            tc.tile_pool(name="proj_psum", bufs=3, space="PSUM") as proj_psum,
            tc.tile_pool(name="tp_psum", bufs=2, space="PSUM") as tp_psum,
        ):
            xT_r = xT.rearrange("(a p) t -> p a t", p=128)  # [128, 8, TK]
            ve_r = ve.rearrange("(a p) d -> p a d", p=128)  # [128, 20, 256]
            TB = 512  # t rows per x block load
            for tb in range(TK // TB):
                xt = xt_pool.tile([128, 8, TB], F32R)
                # split into two DMAs so compute can start on the first half;
                # on the first block interleave the weight loads between the
                # x halves so the first matmul starts as early as possible
                xsrc = xT_r[:, :, tb * TB:(tb + 1) * TB].bitcast(F32R)
                nc.sync.dma_start(out=xt[:, 0:4, :], in_=xsrc[:, 0:4, :])
                if tb == 0:
                    for dch in range(4):
                        nc.sync.dma_start(out=w_sb[:, dch, :],
                                          in_=wq_r[:, dch, :])
                nc.sync.dma_start(out=xt[:, 4:8, :], in_=xsrc[:, 4:8, :])
                vet = ve_pool.tile([128, 4, HPC * HD], F32)
                nc.sync.dma_start(out=vet, in_=ve_r[:, tb * 4:(tb + 1) * 4, :])
                if tb == 0:
                    # bulk loads deferred behind the first x block
                    for dch in range(4, 8):
                        nc.sync.dma_start(out=w_sb[:, dch, :], in_=wq_r[:, dch, :])
                    nc.sync.dma_start(
                        out=cos_sb, in_=cosT.rearrange("(a p) f -> p a f", p=128))
                    nc.sync.dma_start(
                        out=sin_sb, in_=sinT.rearrange("(a p) f -> p a f", p=128))
                    nc.sync.dma_start(out=lam_sb, in_=lam[:])
                for tsub in range(TB // 128):
                    c = tb * (TB // 128) + tsub  # t-chunk index, 0..19
                    psum = proj_psum.tile([128, EW], F32)
                    for dch in range(8):
                        lhsT = xt[:, dch, tsub * 128:(tsub + 1) * 128]
                        if c >= 4:
                            nc.tensor.matmul(
                                psum[:, 0:512], lhsT, w_sb[:, dch, 0:512],
                                start=(dch == 0), stop=(dch == 7),
                            )
                        else:  # halo rows need only k,v
                            nc.tensor.matmul(
                                psum[:, 256:512], lhsT, w_sb[:, dch, 256:512],
                                start=(dch == 0), stop=(dch == 7),
                            )
                        nc.tensor.matmul(
                            psum[:, 512:EW], lhsT, w_sb[:, dch, 512:EW],
                            start=(dch == 0), stop=(dch == 7),
                        )
                    # psum segments: q0 q1 k0 k1 v0 v1, each [128, 128]
                    psum6 = psum.rearrange("p (s d) -> p s d", s=6)

                    # RMS-norm scales (halo chunks skip the q segments).
                    # Square on ACT with fused per-segment row-sum accumulation.
                    s0 = 0 if c >= 4 else 2
                    sq = stageA.tile([128, 6, HD], F32)
                    ssum = smallA.tile([128, 6], F32)
                    for sg in range(s0, 6):
                        nc.scalar.activation(
                            sq[:, sg, :], psum6[:, sg, :], AF.Square,
                            accum_out=ssum[:, sg:sg + 1],
                        )
                    # rms for q,k (eps bias) and v (lam0 folded via scale/bias)
                    rms = smallA.tile([128, 6], F32)
                    nc.scalar.activation(rms[:, s0:4], ssum[:, s0:4], AF.Sqrt,
                                         bias=eps_sb, scale=1.0 / HD)
                    nc.scalar.activation(rms[:, 4:6], ssum[:, 4:6], AF.Sqrt,
                                         bias=lam_sb[:, 3:4],
                                         scale=lam_sb[:, 2:3])
                    rs = smallA.tile([128, 6], F32)
                    nc.vector.reciprocal(rs[:, s0:6], rms[:, s0:6])

                    # normalize segments in one DVE op -> staging (f32r)
                    st6 = stageA.tile([128, 6, HD], F32R)
                    nc.vector.tensor_tensor(
                        out=st6[:, s0:6, :], in0=psum6[:, s0:6, :],
                        in1=rs[:, s0:6, None].to_broadcast([128, 6 - s0, HD]),
                        op=AOP.mult,
                    )
                    st6f = st6.bitcast(F32)

                    # v = lam1 * ve + v_normed (gpsimd; all-SBUF).
                    # Pool has no TensorScalarPtr, so use two tensor_tensor
                    # ops with a broadcast lam1 operand.
                    vel = stageA.tile([128, 2, HD], F32, name="vel")
                    nc.gpsimd.tensor_tensor(
                        out=vel, in0=vet[:, tsub, :].rearrange("p (h d) -> p h d", h=2),
                        in1=lam_sb[:, 1:2, None].to_broadcast([128, 2, HD]),
                        op=AOP.mult,
                    )
                    for h in range(HPC):
                        nc.vector.tensor_tensor(
                            out=vbf[h][:, c, :], in0=vel[:, h, :],
                            in1=st6f[:, 4 + h, :], op=AOP.add,
                        )

                    # rope on q,k (dims 0:32 rotate with dims 64:96); gpsimd
                    nseg = 4 - s0
                    cs = cos_sb[:, c:c + 1, :].to_broadcast([128, nseg, 32])
                    sn = sin_sb[:, c:c + 1, :].to_broadcast([128, nseg, 32])
                    x1 = st6f[:, s0:4, 0:32]
                    x2 = st6f[:, s0:4, 64:96]
                    t1 = stageA.tile([128, 4, 32], F32)
                    t2 = stageA.tile([128, 4, 32], F32)
                    t3 = stageA.tile([128, 4, 32], F32)
                    t4 = stageA.tile([128, 4, 32], F32)
                    nc.vector.tensor_tensor(out=t1[:, s0:4, :], in0=x1, in1=cs, op=AOP.mult)
                    nc.vector.tensor_tensor(out=t2[:, s0:4, :], in0=x2, in1=sn, op=AOP.mult)
                    nc.gpsimd.tensor_tensor(out=t3[:, s0:4, :], in0=x1, in1=sn, op=AOP.mult)
                    nc.gpsimd.tensor_tensor(out=t4[:, s0:4, :], in0=x2, in1=cs, op=AOP.mult)
                    nc.vector.tensor_add(st6[:, s0:4, 0:32], t1[:, s0:4, :], t2[:, s0:4, :])
                    nc.vector.tensor_sub(st6[:, s0:4, 64:96], t4[:, s0:4, :], t3[:, s0:4, :])

                    # transpose q,k into [dd, t] persistent buffers (f32r)
                    for h in range(HPC):
                        if c >= 4:  # q exists only for own rows
                            tq = tp_psum.tile([128, 128], F32R, name="tq", tag="tp")
                            nc.tensor.transpose(tq, st6[:, h, :], identity)
                            nc.vector.tensor_copy(
                                out=qT[h][:, (c - 4) * 128:(c - 3) * 128], in_=tq)
                        tk = tp_psum.tile([128, 128], F32R, name="tk", tag="tp")
                        nc.tensor.transpose(tk, st6[:, 2 + h, :], identity)
                        nc.vector.tensor_copy(out=kT[h][:, c * 128:(c + 1) * 128],
                                              in_=tk)

        nc.sync.dma_start(
            out=wo_sb, in_=woT.rearrange("(a p) e -> p a e", p=128).bitcast(F32R))
        nc.sync.dma_start(out=pad_r, in_=padcnt.rearrange("(a t) -> a t", a=1))

        # ====== Phase B+C: banded attention with interleaved out-projection ===
        with (
            tc.tile_pool(name="pm_pool", bufs=3) as pm_pool,
            tc.tile_pool(name="smallB", bufs=8) as smallB,
            tc.tile_pool(name="o_out", bufs=4) as o_out,
            tc.tile_pool(name="sc_psum", bufs=3, space="PSUM") as sc_psum,
            tc.tile_pool(name="sum_psum", bufs=1, space="PSUM") as sum_psum,
            tc.tile_pool(name="y_psum", bufs=1, space="PSUM") as y_psum,
            tc.tile_pool(name="o_psum", bufs=2, space="PSUM") as o_psum,
        ):
            def oproj_window(tw):
                # out[:, 512tw:512tw+512] = sum_h woT_h^T @ yT_h window
                for ec in range(8):
                    ops = o_psum.tile([128, 512], F32, name="ops")
                    for h in range(HPC):
                        nc.tensor.matmul(
                            ops,
                            wo_sb[:, h, ec * 128:(ec + 1) * 128],
                            yT[h][:, tw * 512:(tw + 1) * 512],
                            start=(h == 0), stop=(h == HPC - 1),
                            skip_group_check=True,
                        )
                    ot = o_out.tile([128, 512], F32, name="ot")
                    if ec % 2 == 0:
                        nc.scalar.copy(out=ot, in_=ops)
                    else:
                        nc.vector.tensor_copy(out=ot, in_=ops)
                    nc.sync.dma_start(
                        out=outT[ec * 128:(ec + 1) * 128,
                                 tw * 512:(tw + 1) * 512],
                        in_=ot,
                    )

            for pr in range(NPR):
                for h in range(HPC):
                    qs = qT[h][:, pr * 256:(pr + 1) * 256]
                    pm = pm_pool.tile([128, NPC, 256], F32R)
                    sums = sum_psum.tile([1, 256], F32, name="sums")
                    yps = y_psum.tile([128, 256], F32, name="yps")
                    # Issue ALL score matmuls (and their exps) before any
                    # exp-dependent accumulation matmul: the PE then has six
                    # back-to-back score matmuls in flight while the ACT
                    # computes the first exp, instead of stalling ~0.7us on
                    # every chunk pair. sc pool bufs=3 holds the step's three
                    # pairs; masked pairs go first so the accumulation tail
                    # has no Pool mask op on its critical path.
                    for wp in (0, 2, 1):  # chunk pairs
                        sc = sc_psum.tile([128, 2, 256], F32, name="sc", tag="sc")
                        for j in range(2):
                            wc = 2 * wp + j
                            nc.tensor.matmul(
                                sc[:, j, :],
                                kT[h][:, (2 * pr + wc) * 128:(2 * pr + wc + 1) * 128],
                                qs, start=True, stop=True, skip_group_check=True,
                            )
                        nc.scalar.activation(pm[:, 2 * wp:2 * wp + 2, :], sc,
                                             AF.Exp, scale=ATTN_SCALE)
                        if wp == 0:
                            nc.vector.tensor_tensor(
                                out=pm[:, 0:2, :], in0=pm[:, 0:2, :].bitcast(F32),
                                in1=maskA, op=AOP.mult)
                        elif wp == 2:
                            nc.vector.tensor_tensor(
                                out=pm[:, 4:6, :], in0=pm[:, 4:6, :].bitcast(F32),
                                in1=maskB, op=AOP.mult)
                    for i, wp in enumerate((0, 2, 1)):
                        for j in range(2):
                            wc = 2 * wp + j
                            nc.tensor.matmul(
                                sums, ones_col, pm[:, wc, :],
                                start=(i == 0 and j == 0),
                                stop=(i == 2 and j == 1),
                                skip_group_check=True,
                            )
                            nc.tensor.matmul(
                                yps, vbf[h][:, 2 * pr + wc, :], pm[:, wc, :],
                                start=(i == 0 and j == 0),
                                stop=(i == 2 and j == 1),
                                skip_group_check=True,
                            )
                    with tc.high_priority(offset=40):
                        sums2 = smallB.tile([1, 256], F32)
                        nc.vector.tensor_sub(sums2, sums,
                                             pad_r[:, pr * 256:(pr + 1) * 256])
                        recip = smallB.tile([1, 256], F32)
                        nc.vector.reciprocal(recip, sums2)
                        # broadcast 1/sum across partitions on the Pool engine
                        bc_sb = smallB.tile([128, 256], F32, name="bc_sb")
                        nc.gpsimd.partition_broadcast(bc_sb, recip)
                    # evacuate with the 1/sum normalization fused (cast f32r)
                    nc.vector.tensor_tensor(
                        out=yT[h][:, pr * 256:(pr + 1) * 256],
                        in0=yps, in1=bc_sb, op=AOP.mult)
                if pr % 2 == 1:
                    oproj_window(pr // 2)


_NC_CACHE = None


def _get_nc():
    global _NC_CACHE
    if _NC_CACHE is None:
        _NC_CACHE = build_kernel()
    return _NC_CACHE


def _rope_tables(positions):
    keep = HD // 4
    active = (1.0 / ROPE_BASE) ** np.linspace(0.0, 1.0, keep, dtype=np.float32)
    theta = positions[:, None].astype(np.float32) * active[None, :]  # [n, 32]
    return np.cos(theta).astype(np.float32), np.sin(theta).astype(np.float32)


def make_in_maps(x, ve, lambdas, qkvo_w):
    """Build the 8 per-core input maps from full inputs (host-side sharding)."""
    x2 = x.reshape(T, DIM)
    ve2 = ve.reshape(T, DIM)
    qw, kw, vw, ow = qkvo_w[0], qkvo_w[1], qkvo_w[2], qkvo_w[3]

    in_maps = []
    for c in range(8):
        s, g = divmod(c, G)
        h0, h1 = HPC * g, HPC * g + 1
        lo = TC * s - WINDOW  # first k/v row (may be negative -> zero pad)
        hi = TC * s + TC

        # xT slice with zero pad
        xs = np.zeros((TK, DIM), np.float32)
        src_lo = max(lo, 0)
        xs[src_lo - lo:, :] = x2[src_lo:hi, :]
        xTc = np.ascontiguousarray(xs.T)

        # fused qkv weight, transposed: cols = q0 q1 k0 k1 v0 v1
        wcols = []
        for wmat in (qw, kw, vw):
            for h in (h0, h1):
                wcols.append(wmat[h * HD:(h + 1) * HD, :].T)
        wqkvT = np.ascontiguousarray(np.concatenate(wcols, axis=1))

        woT = np.ascontiguousarray(ow[:, h0 * HD:(h1 + 1) * HD].T)

        ves = np.zeros((TK, HPC * HD), np.float32)
        ves[src_lo - lo:, :] = ve2[src_lo:hi, h0 * HD:(h1 + 1) * HD]

        pos = np.clip(np.arange(lo, hi), 0, None)
        cosT, sinT = _rope_tables(pos)

        l0, l1 = float(lambdas[0]), float(lambdas[1])
        lam_row = np.array([l0, l1, 1.0 / (HD * l0 * l0), EPS / (l0 * l0)],
                           np.float32)
        lam = np.tile(lam_row.reshape(1, 4), (128, 1)).astype(np.float32)

        pc = np.zeros(TC, np.float32)
        if s == 0:
            i = np.arange(TC)
            pc = np.maximum(0.0, WINDOW - 1.0 - i).astype(np.float32)

        in_maps.append({
            "xT": xTc, "wqkvT": wqkvT, "woT": woT, "ve": ves,
            "cosT": cosT, "sinT": sinT, "lam": lam, "padcnt": pc,
        })
    return in_maps


def kernel(x, ve, lambdas, qkvo_w, window):
    assert int(window) == WINDOW
    x = np.asarray(x, np.float32)
    ve = np.asarray(ve, np.float32)
    lambdas = np.asarray(lambdas, np.float32)
    qkvo_w = np.asarray(qkvo_w, np.float32)

    nc = _get_nc()
    in_maps = make_in_maps(x, ve, lambdas, qkvo_w)
    res = run_bass_kernel_spmd(nc, in_maps, core_ids=list(range(8)))

    outT_full = np.zeros((DIM, T), np.float32)
    for c in range(8):
        s = c // G
        outT_full[:, TC * s:TC * (s + 1)] += res.results[c]["outT"]
    return np.ascontiguousarray(outT_full.T).reshape(1, T, DIM)


if __name__ == "__main__":
    nc = _get_nc()
    print("kernel built ok")


# revision 45
# speedup vs baseline: 1.0538x; 1.0016x over previous
"""Trainium2 Bass kernel for sliding-window causal self-attention.

Reference computation (B=1, T=4096, H=8 heads, head_dim=128, DIM=1024):
  qkv = x @ w_qkv.T; q,k = rms_norm -> rope; v = lam0*rms_norm(v) + lam1*ve
  scores = (q k^T) * 0.12 with sliding-window causal mask (0 <= i-j < 512)
  y = softmax(scores) @ v;  out = y @ o_w.T

Sharding over 8 cores: 2 sequence halves (S) x 4 head-pair groups (G).
Core c = 4*s + g handles t in [2048s, 2048(s+1)) for heads {2g, 2g+1}.
Each core reads its x rows plus a 512-row halo of preceding rows (for k/v),
computes its partial output projection over its 2 heads, and the host sums
the 4 partials per half and concatenates the halves. No on-chip collectives.

Attention uses a transposeless [kj, qi] scores layout: q-tiles are processed
in pairs (256 queries, 768-key window, 6 key chunks) so every matmul has a
moving free dim >= 256, which is required for full-rate float32r matmuls.
Softmax runs without max-subtraction (scores are bounded by 0.12*128), the
kj-sum is done on the PE with a ones vector, and the reciprocal is broadcast
across partitions with a rank-1 matmul. The output projection is interleaved
into the attention loop so its DMA overlaps compute. Elementwise work is
spread across DVE / ScalarE / GpSimd to keep all engines busy.
"""

import sys

sys.path.insert(0, "/opt/trn_rl_repo")

import numpy as np

import concourse.bass as bass
import concourse.mybir as mybir
import concourse.tile as tile
from concourse import bacc
from concourse.bass_utils import run_bass_kernel_spmd
from concourse.masks import make_identity

# Problem constants
T = 4096
DIM = 1024
H = 8
HD = 128
WINDOW = 512
ATTN_SCALE = 0.12
ROPE_BASE = 1024.0
EPS = 1e-6

# Sharding
S = 2          # sequence halves
G = 4          # head groups (2 heads each)
HPC = H // G   # heads per core = 2
TC = T // S    # own rows per core = 2048
TK = TC + WINDOW  # k/v rows incl. halo = 2560
NQT = TC // 128   # q tiles per head = 16
NKC = TK // 128   # k chunks = 20
NPR = TC // 256   # q pairs per head = 8
PW = 256 + WINDOW  # pair window = 768
NPC = PW // 128    # chunks per pair window = 6
EW = 3 * HPC * HD  # fused qkv width per core = 768

F32 = mybir.dt.float32
F32R = mybir.dt.float32r

AOP = mybir.AluOpType
AF = mybir.ActivationFunctionType


def build_kernel():
    nc = bacc.Bacc()

    # Per-core DRAM I/O (shapes identical across cores; data differs).
    xT = nc.declare_dram_parameter("xT", [DIM, TK], F32, isOutput=False)
    wqkvT = nc.declare_dram_parameter("wqkvT", [DIM, EW], F32, isOutput=False)
    woT = nc.declare_dram_parameter("woT", [HPC * HD, DIM], F32, isOutput=False)
    ve = nc.declare_dram_parameter("ve", [TK, HPC * HD], F32, isOutput=False)
    cosT = nc.declare_dram_parameter("cosT", [TK, 32], F32, isOutput=False)
    sinT = nc.declare_dram_parameter("sinT", [TK, 32], F32, isOutput=False)
    lam = nc.declare_dram_parameter("lam", [128, 4], F32, isOutput=False)
    padcnt = nc.declare_dram_parameter("padcnt", [TC], F32, isOutput=False)
    outT = nc.declare_dram_parameter("outT", [DIM, TC], F32, isOutput=True)

    with tile.TileContext(nc) as tc:
        _trace_body(nc, tc, xT, wqkvT, woT, ve, cosT, sinT, lam, padcnt, outT)

    nc.compile()
    return nc


def _trace_body(nc, tc, xT, wqkvT, woT, ve, cosT, sinT, lam, padcnt, outT):
    import contextlib

    ctx = contextlib.ExitStack()
    with ctx:
        const = ctx.enter_context(tc.tile_pool(name="const", bufs=1))
        persist = ctx.enter_context(tc.tile_pool(name="persist", bufs=1))

        # ---- constants needed by phase A (w split per d-chunk so the
        # first projection matmuls can start early) ----
        w_sb = const.tile([128, 8, EW], F32R)  # wqkvT as [dpart, dchunk, e]
        wq_r = wqkvT.rearrange("(a p) e -> p a e", p=128).bitcast(F32R)
        cos_sb = const.tile([128, NKC, 32], F32)
        sin_sb = const.tile([128, NKC, 32], F32)
        lam_sb = const.tile([128, 4], F32)

        identity = const.tile([128, 128], F32R)
        idf = const.tile([128, 128], F32)
        make_identity(nc, idf)
        nc.vector.tensor_copy(out=identity, in_=idf)

        eps_sb = const.tile([128, 1], F32)
        nc.vector.memset(eps_sb, EPS)

        # ---- B/C constant tiles (DMAs deferred until after phase A) ----
        wo_sb = const.tile([128, HPC, DIM], F32R)  # woT as [ddpart, head, e]
        pad_r = const.tile([1, TC], F32)

        onescf = const.tile([128, 1], F32)
        nc.vector.memset(onescf, 1.0)
        ones_col = const.tile([128, 1], F32R)
        nc.vector.tensor_copy(out=ones_col, in_=onescf)

        # Band masks in [kj, ci, qi] orientation for pair-window chunks.
        # Chunk c of a pair window is valid iff qi+1 <= 128c + kj <= qi+512.
        # Chunks 2,3 are always fully valid; 0,1 need the lower bound and
        # 4,5 the upper bound.
        maskA = const.tile([128, 2, 256], F32)  # chunks 0,1
        nc.gpsimd.memset(maskA, 1.0)
        nc.gpsimd.affine_select(
            out=maskA, in_=maskA, compare_op=AOP.is_ge, fill=0.0,
            base=-1, channel_multiplier=1, pattern=[[128, 2], [-1, 256]],
        )
        maskB = const.tile([128, 2, 256], F32)  # chunks 4,5
        nc.gpsimd.memset(maskB, 1.0)
        nc.gpsimd.affine_select(
            out=maskB, in_=maskB, compare_op=AOP.is_ge, fill=0.0,
            base=0, channel_multiplier=-1, pattern=[[-128, 2], [1, 256]],
        )

        # ---- persistent activations ----
        # qT/kT: [dd, t] per head; v: [t(kj) part, chunk, dd]; yT: [dd, t].
        qT = [persist.tile([128, TC], F32R, name=f"qT{h}") for h in range(HPC)]
        kT = [persist.tile([128, TK], F32R, name=f"kT{h}") for h in range(HPC)]
        vbf = [persist.tile([128, NKC, HD], F32R, name=f"vbf{h}") for h in range(HPC)]
        yT = [persist.tile([128, TC], F32R, name=f"yT{h}") for h in range(HPC)]

        # ================= Phase A: QKV projection + norm/rope =================
        with (
            tc.tile_pool(name="xt_pool", bufs=2) as xt_pool,
            tc.tile_pool(name="ve_pool", bufs=2) as ve_pool,
            tc.tile_pool(name="stageA", bufs=4) as stageA,
            tc.tile_pool(name="smallA", bufs=8) as smallA,
            tc.tile_pool(name="proj_psum", bufs=3, space="PSUM") as proj_psum,
            tc.tile_pool(name="tp_psum", bufs=2, space="PSUM") as tp_psum,
        ):
            xT_r = xT.rearrange("(a p) t -> p a t", p=128)  # [128, 8, TK]
            ve_r = ve.rearrange("(a p) d -> p a d", p=128)  # [128, 20, 256]
            TB = 512  # t rows per x block load
            for tb in range(TK // TB):
                xt = xt_pool.tile([128, 8, TB], F32R)
                # split into two DMAs so compute can start on the first half;
                # on the first block interleave the weight loads between the
                # x halves so the first matmul starts as early as possible
                xsrc = xT_r[:, :, tb * TB:(tb + 1) * TB].bitcast(F32R)
                nc.sync.dma_start(out=xt[:, 0:4, :], in_=xsrc[:, 0:4, :])
                if tb == 0:
                    for dch in range(4):
                        nc.sync.dma_start(out=w_sb[:, dch, :],
                                          in_=wq_r[:, dch, :])
                nc.sync.dma_start(out=xt[:, 4:8, :], in_=xsrc[:, 4:8, :])
                vet = ve_pool.tile([128, 4, HPC * HD], F32)
                nc.sync.dma_start(out=vet, in_=ve_r[:, tb * 4:(tb + 1) * 4, :])
                if tb == 0:
                    # bulk loads deferred behind the first x block
                    for dch in range(4, 8):
                        nc.sync.dma_start(out=w_sb[:, dch, :], in_=wq_r[:, dch, :])
                    nc.sync.dma_start(
                        out=cos_sb, in_=cosT.rearrange("(a p) f -> p a f", p=128))
                    nc.sync.dma_start(
                        out=sin_sb, in_=sinT.rearrange("(a p) f -> p a f", p=128))
                    nc.sync.dma_start(out=lam_sb, in_=lam[:])
                for tsub in range(TB // 128):
                    c = tb * (TB // 128) + tsub  # t-chunk index, 0..19
                    psum = proj_psum.tile([128, EW], F32)
                    for dch in range(8):
                        lhsT = xt[:, dch, tsub * 128:(tsub + 1) * 128]
                        if c >= 4:
                            nc.tensor.matmul(
                                psum[:, 0:512], lhsT, w_sb[:, dch, 0:512],
                                start=(dch == 0), stop=(dch == 7),
                            )
                        else:  # halo rows need only k,v
                            nc.tensor.matmul(
                                psum[:, 256:512], lhsT, w_sb[:, dch, 256:512],
                                start=(dch == 0), stop=(dch == 7),
                            )
                        nc.tensor.matmul(
                            psum[:, 512:EW], lhsT, w_sb[:, dch, 512:EW],
                            start=(dch == 0), stop=(dch == 7),
                        )
                    # psum segments: q0 q1 k0 k1 v0 v1, each [128, 128]
                    psum6 = psum.rearrange("p (s d) -> p s d", s=6)

                    # RMS-norm scales (halo chunks skip the q segments).
                    # Square on ACT with fused per-segment row-sum accumulation.
                    s0 = 0 if c >= 4 else 2
                    sq = stageA.tile([128, 6, HD], F32)
                    ssum = smallA.tile([128, 6], F32)
                    for sg in range(s0, 6):
                        nc.scalar.activation(
                            sq[:, sg, :], psum6[:, sg, :], AF.Square,
                            accum_out=ssum[:, sg:sg + 1],
                        )
                    # rms for q,k (eps bias) and v (lam0 folded via scale/bias)
                    rms = smallA.tile([128, 6], F32)
                    nc.scalar.activation(rms[:, s0:4], ssum[:, s0:4], AF.Sqrt,
                                         bias=eps_sb, scale=1.0 / HD)
                    nc.scalar.activation(rms[:, 4:6], ssum[:, 4:6], AF.Sqrt,
                                         bias=lam_sb[:, 3:4],
                                         scale=lam_sb[:, 2:3])
                    rs = smallA.tile([128, 6], F32)
                    nc.vector.reciprocal(rs[:, s0:6], rms[:, s0:6])

                    # normalize segments in one DVE op -> staging (f32r)
                    st6 = stageA.tile([128, 6, HD], F32R)
                    nc.vector.tensor_tensor(
                        out=st6[:, s0:6, :], in0=psum6[:, s0:6, :],
                        in1=rs[:, s0:6, None].to_broadcast([128, 6 - s0, HD]),
                        op=AOP.mult,
                    )
                    st6f = st6.bitcast(F32)

                    # v = lam1 * ve + v_normed (gpsimd; all-SBUF).
                    # Pool has no TensorScalarPtr, so use two tensor_tensor
                    # ops with a broadcast lam1 operand.
                    vel = stageA.tile([128, 2, HD], F32, name="vel")
                    nc.gpsimd.tensor_tensor(
                        out=vel, in0=vet[:, tsub, :].rearrange("p (h d) -> p h d", h=2),
                        in1=lam_sb[:, 1:2, None].to_broadcast([128, 2, HD]),
                        op=AOP.mult,
                    )
                    for h in range(HPC):
                        nc.vector.tensor_tensor(
                            out=vbf[h][:, c, :], in0=vel[:, h, :],
                            in1=st6f[:, 4 + h, :], op=AOP.add,
                        )

                    # rope on q,k (dims 0:32 rotate with dims 64:96); gpsimd
                    nseg = 4 - s0
                    cs = cos_sb[:, c:c + 1, :].to_broadcast([128, nseg, 32])
                    sn = sin_sb[:, c:c + 1, :].to_broadcast([128, nseg, 32])
                    x1 = st6f[:, s0:4, 0:32]
                    x2 = st6f[:, s0:4, 64:96]
                    t1 = stageA.tile([128, 4, 32], F32)
                    t2 = stageA.tile([128, 4, 32], F32)
                    t3 = stageA.tile([128, 4, 32], F32)
                    t4 = stageA.tile([128, 4, 32], F32)
                    nc.vector.tensor_tensor(out=t1[:, s0:4, :], in0=x1, in1=cs, op=AOP.mult)
                    nc.vector.tensor_tensor(out=t2[:, s0:4, :], in0=x2, in1=sn, op=AOP.mult)
                    nc.gpsimd.tensor_tensor(out=t3[:, s0:4, :], in0=x1, in1=sn, op=AOP.mult)
                    nc.gpsimd.tensor_tensor(out=t4[:, s0:4, :], in0=x2, in1=cs, op=AOP.mult)
                    nc.vector.tensor_add(st6[:, s0:4, 0:32], t1[:, s0:4, :], t2[:, s0:4, :])
                    nc.vector.tensor_sub(st6[:, s0:4, 64:96], t4[:, s0:4, :], t3[:, s0:4, :])

                    # transpose q,k into [dd, t] persistent buffers (f32r)
                    for h in range(HPC):
                        if c >= 4:  # q exists only for own rows
                            tq = tp_psum.tile([128, 128], F32R, name="tq", tag="tp")
                            nc.tensor.transpose(tq, st6[:, h, :], identity)
                            nc.vector.tensor_copy(
                                out=qT[h][:, (c - 4) * 128:(c - 3) * 128], in_=tq)
                        tk = tp_psum.tile([128, 128], F32R, name="tk", tag="tp")
                        nc.tensor.transpose(tk, st6[:, 2 + h, :], identity)
                        nc.vector.tensor_copy(out=kT[h][:, c * 128:(c + 1) * 128],
                                              in_=tk)

        nc.sync.dma_start(
            out=wo_sb, in_=woT.rearrange("(a p) e -> p a e", p=128).bitcast(F32R))
        nc.sync.dma_start(out=pad_r, in_=padcnt.rearrange("(a t) -> a t", a=1))

        # ====== Phase B+C: banded attention with interleaved out-projection ===
        with (
            tc.tile_pool(name="pm_pool", bufs=3) as pm_pool,
            tc.tile_pool(name="smallB", bufs=8) as smallB,
            tc.tile_pool(name="o_out", bufs=4) as o_out,
            tc.tile_pool(name="sc_psum", bufs=4, space="PSUM") as sc_psum,
            tc.tile_pool(name="sum_psum", bufs=1, space="PSUM") as sum_psum,
            tc.tile_pool(name="y_psum", bufs=1, space="PSUM") as y_psum,
            tc.tile_pool(name="o_psum", bufs=2, space="PSUM") as o_psum,
        ):
            def oproj_window(tw):
                # out[:, 512tw:512tw+512] = sum_h woT_h^T @ yT_h window
                for ec in range(8):
                    ops = o_psum.tile([128, 512], F32, name="ops")
                    for h in range(HPC):
                        nc.tensor.matmul(
                            ops,
                            wo_sb[:, h, ec * 128:(ec + 1) * 128],
                            yT[h][:, tw * 512:(tw + 1) * 512],
                            start=(h == 0), stop=(h == HPC - 1),
                            skip_group_check=True,
                        )
                    ot = o_out.tile([128, 512], F32, name="ot")
                    if ec % 2 == 0:
                        nc.scalar.copy(out=ot, in_=ops)
                    else:
                        nc.vector.tensor_copy(out=ot, in_=ops)
                    nc.sync.dma_start(
                        out=outT[ec * 128:(ec + 1) * 128,
                                 tw * 512:(tw + 1) * 512],
                        in_=ot,
                    )

            # Cross-step software pipeline: step n+1's six score matmuls
            # (and their exps/masks) are issued before step n's twelve
            # accumulation matmuls, so the exp chain of the next step runs
            # on ACT/DVE while the PE drains the current accumulation.
            def issue_step(pr, h):
                qs = qT[h][:, pr * 256:(pr + 1) * 256]
                pm = pm_pool.tile([128, NPC, 256], F32R, name="pm")
                for wp in (0, 2, 1):  # chunk pairs, masked pairs first
                    sc = sc_psum.tile([128, 2, 256], F32, name="sc", tag="sc")
                    for j in range(2):
                        wc = 2 * wp + j
                        nc.tensor.matmul(
                            sc[:, j, :],
                            kT[h][:, (2 * pr + wc) * 128:(2 * pr + wc + 1) * 128],
                            qs, start=True, stop=True, skip_group_check=True,
                        )
                    nc.scalar.activation(pm[:, 2 * wp:2 * wp + 2, :], sc,
                                         AF.Exp, scale=ATTN_SCALE)
                    if wp == 0:
                        nc.vector.tensor_tensor(
                            out=pm[:, 0:2, :], in0=pm[:, 0:2, :].bitcast(F32),
                            in1=maskA, op=AOP.mult)
                    elif wp == 2:
                        nc.vector.tensor_tensor(
                            out=pm[:, 4:6, :], in0=pm[:, 4:6, :].bitcast(F32),
                            in1=maskB, op=AOP.mult)
                return pm

            def consume_step(pr, h, pm):
                sums = sum_psum.tile([1, 256], F32, name="sums")
                yps = y_psum.tile([128, 256], F32, name="yps")
                for i, wp in enumerate((0, 2, 1)):
                    for j in range(2):
                        wc = 2 * wp + j
                        nc.tensor.matmul(
                            sums, ones_col, pm[:, wc, :],
                            start=(i == 0 and j == 0),
                            stop=(i == 2 and j == 1),
                            skip_group_check=True,
                        )
                        nc.tensor.matmul(
                            yps, vbf[h][:, 2 * pr + wc, :], pm[:, wc, :],
                            start=(i == 0 and j == 0),
                            stop=(i == 2 and j == 1),
                            skip_group_check=True,
                        )
                with tc.high_priority(offset=40):
                    sums2 = smallB.tile([1, 256], F32)
                    nc.vector.tensor_sub(sums2, sums,
                                         pad_r[:, pr * 256:(pr + 1) * 256])
                    recip = smallB.tile([1, 256], F32)
                    nc.vector.reciprocal(recip, sums2)
                    # broadcast 1/sum across partitions on the Pool engine
                    bc_sb = smallB.tile([128, 256], F32, name="bc_sb")
                    nc.gpsimd.partition_broadcast(bc_sb, recip)
                # evacuate with the 1/sum normalization fused (cast f32r)
                nc.vector.tensor_tensor(
                    out=yT[h][:, pr * 256:(pr + 1) * 256],
                    in0=yps, in1=bc_sb, op=AOP.mult)
                if h == HPC - 1 and pr % 2 == 1:
                    oproj_window(pr // 2)

            steps = [(pr, h) for pr in range(NPR) for h in range(HPC)]
            pms = [issue_step(*steps[0])]
            for i, (pr, h) in enumerate(steps):
                if i + 1 < len(steps):
                    pms.append(issue_step(*steps[i + 1]))
                consume_step(pr, h, pms[i])
                pms[i] = None


_NC_CACHE = None


def _get_nc():
    global _NC_CACHE
    if _NC_CACHE is None:
        _NC_CACHE = build_kernel()
    return _NC_CACHE


def _rope_tables(positions):
    keep = HD // 4
    active = (1.0 / ROPE_BASE) ** np.linspace(0.0, 1.0, keep, dtype=np.float32)
    theta = positions[:, None].astype(np.float32) * active[None, :]  # [n, 32]
    return np.cos(theta).astype(np.float32), np.sin(theta).astype(np.float32)


def make_in_maps(x, ve, lambdas, qkvo_w):
    """Build the 8 per-core input maps from full inputs (host-side sharding)."""
    x2 = x.reshape(T, DIM)
    ve2 = ve.reshape(T, DIM)
    qw, kw, vw, ow = qkvo_w[0], qkvo_w[1], qkvo_w[2], qkvo_w[3]

    in_maps = []
    for c in range(8):
        s, g = divmod(c, G)
        h0, h1 = HPC * g, HPC * g + 1
        lo = TC * s - WINDOW  # first k/v row (may be negative -> zero pad)
        hi = TC * s + TC

        # xT slice with zero pad
        xs = np.zeros((TK, DIM), np.float32)
        src_lo = max(lo, 0)
        xs[src_lo - lo:, :] = x2[src_lo:hi, :]
        xTc = np.ascontiguousarray(xs.T)

        # fused qkv weight, transposed: cols = q0 q1 k0 k1 v0 v1
        wcols = []
        for wmat in (qw, kw, vw):
            for h in (h0, h1):
                wcols.append(wmat[h * HD:(h + 1) * HD, :].T)
        wqkvT = np.ascontiguousarray(np.concatenate(wcols, axis=1))

        woT = np.ascontiguousarray(ow[:, h0 * HD:(h1 + 1) * HD].T)

        ves = np.zeros((TK, HPC * HD), np.float32)
        ves[src_lo - lo:, :] = ve2[src_lo:hi, h0 * HD:(h1 + 1) * HD]

        pos = np.clip(np.arange(lo, hi), 0, None)
        cosT, sinT = _rope_tables(pos)

        l0, l1 = float(lambdas[0]), float(lambdas[1])
        lam_row = np.array([l0, l1, 1.0 / (HD * l0 * l0), EPS / (l0 * l0)],
                           np.float32)
        lam = np.tile(lam_row.reshape(1, 4), (128, 1)).astype(np.float32)

        pc = np.zeros(TC, np.float32)
        if s == 0:
            i = np.arange(TC)
            pc = np.maximum(0.0, WINDOW - 1.0 - i).astype(np.float32)

        in_maps.append({
            "xT": xTc, "wqkvT": wqkvT, "woT": woT, "ve": ves,
            "cosT": cosT, "sinT": sinT, "lam": lam, "padcnt": pc,
        })
    return in_maps


def kernel(x, ve, lambdas, qkvo_w, window):
    assert int(window) == WINDOW
    x = np.asarray(x, np.float32)
    ve = np.asarray(ve, np.float32)
    lambdas = np.asarray(lambdas, np.float32)
    qkvo_w = np.asarray(qkvo_w, np.float32)

    nc = _get_nc()
    in_maps = make_in_maps(x, ve, lambdas, qkvo_w)
    res = run_bass_kernel_spmd(nc, in_maps, core_ids=list(range(8)))

    outT_full = np.zeros((DIM, T), np.float32)
    for c in range(8):
        s = c // G
        outT_full[:, TC * s:TC * (s + 1)] += res.results[c]["outT"]
    return np.ascontiguousarray(outT_full.T).reshape(1, T, DIM)


if __name__ == "__main__":
    nc = _get_nc()
    print("kernel built ok")
